# revision 13
# baseline (speedup 1.0000x reference)
"""GQA attention (b=2, s=2048, d=2048, H=16, Hkv=4, depth=128) on 8 trn2 cores.

Sharding: core c = 4*b + j (b in {0,1}, j in {0..3}) handles batch b and
q-heads {2j, 2j+1, 2j+8, 2j+9}.  This model's RoPE rotates the full projected
vector (pairing dim i with i + d/2), so roped q-head h mixes raw column
blocks {h mod 8, (h mod 8) + 8}; the head grouping above makes the Wq column
shard exactly 512 columns with no duplication.  Those q-heads attend kv-heads
{g0, g0+2} (g0 = 0 for j<2 else 1), which likewise pair up under RoPE.
Each core of a pair projects ONE raw k block and ONE v head; the pair swaps
them with a 2-way AllGather, halving the duplicated K/V projection work.
Wo is row-sharded over the 4 local head-dims; the 4 per-batch bf16 partials
are summed on the host (fp32) and bo added.

Device layout is fully transposed (feature dim on partitions): q_r^T, k_r^T
are [depth, s]; logits are computed as l^T = k_r^T.T @ q_r^T so the softmax
free axis is sq and the PV matmul needs no transposes (v is kept native
[s, dv] via on-chip DMA transposes).  All matmuls run in bf16 (fp32 PSUM).

Schedule (the PE is in-order, so overlap must be arranged at emission time):
attention runs sq-tile-major; each (st, head) does 8 iterations of
{2 QK matmuls -> one exp ACT over a [128,1024] 2-bank PSUM pair -> 2 PV
matmuls}.  The exp ACT (1114ns) outweighs the 4 matmuls (864ns), so the
output-projection matmuls of the PREVIOUS sq tile are interleaved between
iterations to keep the PE saturated (and its DVFS p-state at 2.4 GHz).
Softmax denominators are a bf16 running sum on DVE over the exp tiles, a
cross-partition ones-matmul reduce + broadcast on PE, and a DVE divide that
writes normalized o^T directly; the ACT engine does nothing but exp (one
activation table, loaded once).  PSUM drains run on ACT during the
projection phase and on DVE elsewhere (GPSIMD cannot touch PSUM).  Inputs stream per k-chunk so projection accumulation pipelines
with the DMAs; weights load as single DMAs to cut semaphore waits.
"""
import numpy as np
import ml_dtypes
from contextlib import ExitStack

import concourse.bass as bass
import concourse.mybir as mybir
import concourse.tile as tile
from concourse.bass import ts
from concourse.bass_utils import run_bass_kernel_spmd

BF = mybir.dt.bfloat16
F32 = mybir.dt.float32
NPBF = ml_dtypes.bfloat16

S = 2048          # sequence length
D = 2048          # d_model
DEPTH = 128       # head dim
NKC = 16          # contraction chunks of 128 over d_model
NST = 4           # 512-wide s tiles
INV_SQRT_D = 1.0 / float(np.sqrt(np.float32(DEPTH)))

_NC_CACHE = None
LAST_RESULT = None  # BassKernelResults of the most recent run (for profiling)


def _split_waits(nc, limit=1, dma_limit=1):
    """walrus rejects instructions carrying more than a couple of sem waits
    ('Too many sync wait commands'; DMA descriptors take only one).  Move
    excess waits onto dedicated NoOps on the same engine, placed immediately
    before the instruction."""
    idx = 0
    for f in nc.m.functions:
        for blk in f.blocks:
            insts = blk.instructions
            out = []
            for inst in insts:
                si = inst.sync_info
                lim = dma_limit if "DMA" in type(inst).__name__.upper() or \
                    "Dma" in type(inst).__name__ else limit
                if si is not None and len(si.on_wait) > lim:
                    waits = list(si.on_wait)
                    extra, keep = waits[:-lim], waits[-lim:]
                    for w in extra:
                        nop = mybir.InstNoOp(name=f"waitsplit_{idx}", ins=[], outs=[])
                        idx += 1
                        nop.engine = inst.engine
                        nop.bass_nofuse = True
                        nop.sync_info = mybir.SyncInfo(on_wait=[w], on_update=[])
                        out.append(nop)
                    inst.sync_info = mybir.SyncInfo(
                        on_wait=keep, on_update=list(si.on_update)
                    )
                out.append(inst)
            insts[:] = out


def _build_nc():
    nc = bass.Bass(num_devices=8)
    xT = nc.dram_tensor("xT", [128, NKC, S], BF, kind="ExternalInput")
    wq = nc.dram_tensor("wq", [128, NKC, 512], BF, kind="ExternalInput")
    wk = nc.dram_tensor("wk", [128, NKC, 128], BF, kind="ExternalInput")
    wv = nc.dram_tensor("wv", [128, NKC, 128], BF, kind="ExternalInput")
    wo = nc.dram_tensor("wo", [128, 4, D], BF, kind="ExternalInput")
    cq = nc.dram_tensor("cq", [128, 2, S], BF, kind="ExternalInput")
    sq = nc.dram_tensor("sq", [128, 2, S], BF, kind="ExternalInput")
    ck = nc.dram_tensor("ck", [128, S], BF, kind="ExternalInput")
    sk = nc.dram_tensor("sk", [128, S], BF, kind="ExternalInput")
    out = nc.dram_tensor("out", [128, 16, D], BF, kind="ExternalOutput")

    with tile.TileContext(nc) as tc, ExitStack() as top:
        pool_p = top.enter_context(tc.tile_pool(name="persist", bufs=1))
        # PSUM budget (8 banks): 4 banks for [128,1024] logit pairs /
        # projection pair-accs (tag psb, bufs=2), 1 bank for the PV
        # accumulator (tag pv, bufs=1), 2 banks for out-proj + KV proj accs
        # (tag ob, bufs=2), 1 bank for the denominator reduce + broadcast
        # (tag dn, sequential reuse).
        pp_big = top.enter_context(tc.tile_pool(name="psbig", bufs=2, space="PSUM"))
        pp_pv = top.enter_context(tc.tile_pool(name="pspv", bufs=1, space="PSUM"))
        pp_ob = top.enter_context(tc.tile_pool(name="psob", bufs=2, space="PSUM"))
        pp_dn = top.enter_context(tc.tile_pool(name="psdn", bufs=1, space="PSUM"))

        qr = pool_p.tile([128, 4, S], BF)        # roped qT, slots [a0,a1,a0+8,a1+8]
        kr = pool_p.tile([128, 2, S], BF)        # roped kT,  slots [g0, g0+2]
        vn = pool_p.tile([128, 2, NKC, DEPTH], BF)  # v native [p, g, skc, dv]
        wo_sb = pool_p.tile([128, 4, D], BF)
        ones_col_b = pool_p.tile([128, 1], BF)
        ones_row_b = pool_p.tile([1, 128], BF)
        nc.vector.memset(ones_col_b[:], 1.0)
        nc.vector.memset(ones_row_b[:], 1.0)

        # ---------------- phase 1: projections + rope -----------------
        with ExitStack() as p1:
            pool_x = p1.enter_context(tc.tile_pool(name="p1x", bufs=16))
            pool_w = p1.enter_context(tc.tile_pool(name="p1w", bufs=1))
            pool_tab = p1.enter_context(tc.tile_pool(name="p1t", bufs=1))
            pool_t = p1.enter_context(tc.tile_pool(name="p1tmp", bufs=3))
            pool_vt = p1.enter_context(tc.tile_pool(name="p1vt", bufs=1))
            pool_dram = p1.enter_context(tc.tile_pool(name="p1dram", bufs=1, space="DRAM"))

            # weights in single DMAs (fewer semaphores); x streams per chunk
            wk_sb = pool_w.tile([128, NKC, 128], BF)
            nc.sync.dma_start(wk_sb[:], wk[:])
            wv_sb = pool_w.tile([128, NKC, 128], BF)
            nc.sync.dma_start(wv_sb[:], wv[:])
            xTs = []
            for kc in range(NKC):
                xt_t = pool_x.tile([128, S], BF, tag="xt", name=f"xt_{kc}")
                nc.sync.dma_start(xt_t[:], xT[:, kc, :])
                xTs.append(xt_t)
            wq_sb = pool_w.tile([128, NKC, 512], BF)
            nc.sync.dma_start(wq_sb[:], wq[:])
            nc.sync.dma_start(wo_sb[:], wo[:])
            cq_sb = pool_tab.tile([128, 2, S], BF)
            sq_sb = pool_tab.tile([128, 2, S], BF)
            ck_sb = pool_tab.tile([128, S], BF)
            sk_sb = pool_tab.tile([128, S], BF)
            nc.sync.dma_start(ck_sb[:], ck[:])
            nc.sync.dma_start(sk_sb[:], sk[:])
            nc.sync.dma_start(cq_sb[:], cq[:])
            nc.sync.dma_start(sq_sb[:], sq[:])

            # K/V: each core of a pair projects ONE raw k block and ONE v
            # head; the pair exchanges them with an AllGather, then ropes /
            # transposes locally.  Halves the duplicated K/V projection work.
            kv_sb = pool_vt.tile([128, 2 * S], BF, tag="kvmine")
            for part, w_sel in ((0, wk_sb), (1, wv_sb)):
                for st in range(NST):
                    acc = pp_ob.tile([128, 512], F32, tag="ob")
                    for kc in range(NKC):
                        nc.tensor.matmul(
                            acc[:],
                            w_sel[:, kc, :],
                            xTs[kc][:, ts(st, 512)],
                            start=(kc == 0),
                            stop=(kc == NKC - 1),
                        )
                    nc.scalar.copy(kv_sb[:, ts(part * NST + st, 512)], acc[:])
            kv_in = pool_dram.tile([128, 2 * S], BF)
            kv_out = pool_dram.tile([2, 128, 2 * S], BF)
            nc.sync.dma_start(kv_in[:], kv_sb[:])
            nc.gpsimd.collective_compute(
                "AllGather",
                mybir.AluOpType.bypass,
                replica_groups=[[0, 1], [2, 3], [4, 5], [6, 7]],
                ins=[kv_in.opt()],
                outs=[kv_out.opt()],
            )
            kboth = pool_vt.tile([128, 2, S], BF, tag="kboth")
            vtboth = pool_vt.tile([128, 2, S], BF, tag="vtboth")
            for r in range(2):
                nc.sync.dma_start(kboth[:, r, :], kv_out[r, :, 0:S])
                nc.sync.dma_start(vtboth[:, r, :], kv_out[r, :, S:2 * S])

            # Q: raw blocks (i, 2+i) projected together into one 2-bank
            # PSUM pair, drained by a single Pool copy, roped on DVE.
            for i in range(2):
                for st in range(NST):
                    acc2 = pp_big.tile([128, 1024], F32, tag="psb")
                    for xb in range(2):
                        blk = i if xb == 0 else 2 + i
                        for kc in range(NKC):
                            nc.tensor.matmul(
                                acc2[:, ts(xb, 512)],
                                wq_sb[:, kc, ts(blk, 128)],
                                xTs[kc][:, ts(st, 512)],
                                start=(kc == 0),
                                stop=(kc == NKC - 1),
                            )
                    raw2 = pool_t.tile([128, 1024], BF, tag="raw")
                    nc.scalar.copy(raw2[:], acc2[:])
                    x1, x2 = raw2[:, 0:512], raw2[:, 512:1024]
                    c_ap = cq_sb[:, i, ts(st, 512)]
                    s_ap = sq_sb[:, i, ts(st, 512)]
                    t1 = pool_t.tile([128, 512], BF, tag="t1")
                    t2 = pool_t.tile([128, 512], BF, tag="t2")
                    nc.vector.tensor_mul(t1[:], x1, c_ap)
                    nc.vector.tensor_mul(t2[:], x2, s_ap)
                    nc.vector.tensor_sub(qr[:, i, ts(st, 512)], t1[:], t2[:])
                    t3 = pool_t.tile([128, 512], BF, tag="t1")
                    t4 = pool_t.tile([128, 512], BF, tag="t2")
                    nc.vector.tensor_mul(t3[:], x2, c_ap)
                    nc.vector.tensor_mul(t4[:], x1, s_ap)
                    nc.vector.tensor_add(qr[:, 2 + i, ts(st, 512)], t3[:], t4[:])

            # k rope from the gathered raw blocks (x1 = even core's block g0,
            # x2 = odd core's block g0+2)
            for st in range(NST):
                sl = ts(st, 512)
                x1, x2 = kboth[:, 0, sl], kboth[:, 1, sl]
                c_ap, s_ap = ck_sb[:, sl], sk_sb[:, sl]
                t1 = pool_t.tile([128, 512], BF, tag="t1")
                t2 = pool_t.tile([128, 512], BF, tag="t2")
                nc.vector.tensor_mul(t1[:], x1, c_ap)
                nc.vector.tensor_mul(t2[:], x2, s_ap)
                nc.vector.tensor_sub(kr[:, 0, sl], t1[:], t2[:])
                t3 = pool_t.tile([128, 512], BF, tag="t1")
                t4 = pool_t.tile([128, 512], BF, tag="t2")
                nc.vector.tensor_mul(t3[:], x2, c_ap)
                nc.vector.tensor_mul(t4[:], x1, s_ap)
                nc.vector.tensor_add(kr[:, 1, sl], t3[:], t4[:])

            # v native via DMA transpose
            for g in range(2):
                for skt in range(NKC):
                    nc.sync.dma_start_transpose(
                        vn[:, g, skt, :], vtboth[:, g, ts(skt, 128)]
                    )

        # ------------- phase 2: attention + output projection -------------
        with ExitStack() as p2:
            pool_exp = p2.enter_context(tc.tile_pool(name="exp", bufs=4))
            pool_sum = p2.enter_context(tc.tile_pool(name="sums", bufs=2))
            pool_pr = p2.enter_context(tc.tile_pool(name="pairs", bufs=3))
            pool_on = p2.enter_context(tc.tile_pool(name="onorm", bufs=2))
            pool_sm = p2.enter_context(tc.tile_pool(name="small", bufs=4))
            pool_osb = p2.enter_context(tc.tile_pool(name="osb", bufs=2))

            osb_state = {}

            def emit_outproj_group(onorm_prev, mprev, mi, ct):
                """4 accumulating matmuls of out[:, mprev, ct-block] over the
                4 local heads, drained to o_sb on DVE."""
                if ct == 0:
                    osb_state[mprev] = pool_osb.tile([128, D], BF, tag="osb",
                                                     name=f"osb_{mprev}")
                ob = pp_ob.tile([128, 512], F32, tag="ob", name=f"ob_{mprev}_{ct}")
                for hi2 in range(4):
                    nc.tensor.matmul(
                        ob[:],
                        onorm_prev[:, hi2, ts(mi, 128)],
                        wo_sb[:, hi2, ts(ct, 512)],
                        start=(hi2 == 0),
                        stop=(hi2 == 3),
                    )
                nc.vector.tensor_copy(osb_state[mprev][:, ts(ct, 512)], ob[:])
                if ct == 3:
                    nc.sync.dma_start(out[:, mprev, :], osb_state.pop(mprev)[:])

            def emit_den_steps(pend, upto):
                """Deferred denominator chain of the previous head: runs with
                ~a full head of slack so the PE never waits on the Pool-engine
                fold chain."""
                while pend["step"] < upto:
                    s = pend["step"]
                    pend["step"] += 1
                    st_, hi_, fsum_, o_raw_, onorm_ = (
                        pend["st"], pend["hi"], pend["fsum"], pend["o_raw"],
                        pend["onorm"])
                    if s == 0:
                        sden = pool_sm.tile([128, 512], BF, tag="sd")
                        nc.vector.tensor_add(sden[:], fsum_[:, 0:512],
                                             fsum_[:, 512:1024])
                        den = pp_dn.tile([1, 512], F32, tag="dn",
                                         name=f"dn_{st_}_{hi_}")
                        nc.tensor.matmul(den[:], ones_col_b[:], sden[:],
                                         start=True, stop=True)
                        pend["den"] = den
                    elif s == 1:
                        lnd = pool_sm.tile([1, 512], F32, tag="db")
                        nc.scalar.activation(lnd[:], pend["den"][:],
                                             mybir.ActivationFunctionType.Ln)
                        rec = pool_sm.tile([1, 512], BF, tag="db")
                        nc.scalar.activation(rec[:], lnd[:],
                                             mybir.ActivationFunctionType.Exp,
                                             scale=-1.0)
                        pend["rec"] = rec
                    elif s == 2:
                        bc = pp_dn.tile([128, 512], F32, tag="dn",
                                        name=f"bc_{st_}_{hi_}")
                        nc.tensor.matmul(bc[:], ones_row_b[:], pend["rec"][:],
                                         start=True, stop=True)
                        pend["bc"] = bc
                    elif s == 3:
                        nc.vector.tensor_mul(onorm_[:, hi_, :], o_raw_[:],
                                             pend["bc"][:])

            pending = None       # deferred den chain of the previous head
            op_queue = []        # deferred out-proj groups of the previous st
            onorm_prev = None
            for st in range(NST):
                onorm_st = pool_on.tile([128, 4, 512], BF, tag="on",
                                        name=f"on_{st}")
                for hi in range(4):
                    g = hi // 2
                    o_bank = pp_pv.tile([128, 512], F32, tag="pv",
                                        name=f"opv_{st}_{hi}")
                    fsum = pool_sum.tile([128, 1024], F32, tag="ss",
                                         name=f"fs_{st}_{hi}")
                    e_prev = None
                    p_prev = None
                    for t in range(8):
                        lg = pp_big.tile([128, 1024], F32, tag="psb")
                        nc.tensor.matmul(
                            lg[:, 0:512],
                            kr[:, g, ts(2 * t, 128)],
                            qr[:, hi, ts(st, 512)],
                            start=True, stop=True,
                        )
                        nc.tensor.matmul(
                            lg[:, 512:1024],
                            kr[:, g, ts(2 * t + 1, 128)],
                            qr[:, hi, ts(st, 512)],
                            start=True, stop=True,
                        )
                        e = pool_exp.tile([128, 1024], BF, tag="exp",
                                          name=f"e_{st}_{hi}_{t}")
                        nc.scalar.activation(
                            e[:], lg[:],
                            mybir.ActivationFunctionType.Exp,
                            scale=INV_SQRT_D,
                        )
                        nc.tensor.matmul(
                            o_bank[:], vn[:, g, 2 * t, :], e[:, 0:512],
                            start=(t == 0), stop=False,
                        )
                        nc.tensor.matmul(
                            o_bank[:], vn[:, g, 2 * t + 1, :], e[:, 512:1024],
                            start=False, stop=(t == 7),
                        )
                        # denominator: bf16 pair-adds on DVE, then an f32
                        # fold chain on the otherwise-idle Pool engine
                        # (SBUF-only ops, so Pool is allowed)
                        if t % 2 == 0:
                            e_prev = e
                        else:
                            p = pool_pr.tile([128, 1024], BF, tag="pr",
                                             name=f"p_{st}_{hi}_{t}")
                            nc.vector.tensor_add(p[:], e_prev[:], e[:])
                            if t == 1:
                                p_prev = p
                            elif t == 3:
                                nc.gpsimd.tensor_add(fsum[:], p_prev[:], p[:])
                            else:
                                nc.gpsimd.tensor_add(fsum[:], fsum[:], p[:])
                        # spread the previous head's deferred denominator
                        # steps and the previous sq-tile's output projection
                        # through this head's iterations
                        if pending is not None:
                            emit_den_steps(pending, (1, 2, 2, 3, 3, 4, 4, 4)[t])
                        if op_queue and t in (1, 2, 3, 5, 6, 7):
                            emit_outproj_group(*op_queue.pop(0))
                    if pending is not None:
                        emit_den_steps(pending, 4)
                    # free the PV bank right away (bufs=1); normalization of
                    # this head is deferred into the next head's stream
                    o_raw = pool_sm.tile([128, 512], BF, tag="or")
                    nc.vector.tensor_copy(o_raw[:], o_bank[:])
                    pending = {"step": 0, "st": st, "hi": hi, "fsum": fsum,
                               "o_raw": o_raw, "onorm": onorm_st}
                    if onorm_prev is not None and hi == 0:
                        op_queue = [(onorm_prev, 4 * (st - 1) + mi, mi, ct)
                                    for mi in range(4) for ct in range(4)]
                onorm_prev = onorm_st
            # flush: last head's denominator chain, then the last sq tile's
            # output projection
            emit_den_steps(pending, 4)
            for mi in range(4):
                for ct in range(4):
                    emit_outproj_group(onorm_prev, 12 + mi, mi, ct)

    _split_waits(nc)
    return nc


def _chunk128(arr):
    """(K*128, N) f32 -> [128, K, N] bf16 with [p, k, n] = arr[k*128+p, n]."""
    k = arr.shape[0] // 128
    return np.ascontiguousarray(
        arr.reshape(k, 128, arr.shape[1]).transpose(1, 0, 2)
    ).astype(NPBF)


def _rope_tables(dim):
    pos = np.arange(S, dtype=np.float32)
    inv = (10000.0 ** (-(np.arange(dim, dtype=np.float32)) / np.float32(dim))
           ).astype(np.float32)
    freqs = pos[:, None] * inv[None, :]
    return np.cos(freqs).astype(np.float32), np.sin(freqs).astype(np.float32)


def kernel(x, mask, Wq, Wk, Wv, Wo, bo):
    global _NC_CACHE
    assert np.asarray(mask).all(), "kernel specialized for all-true mask"
    x = np.asarray(x, dtype=np.float32)
    Wq = np.asarray(Wq, dtype=np.float32)
    Wk = np.asarray(Wk, dtype=np.float32)
    Wv = np.asarray(Wv, dtype=np.float32)
    Wo = np.asarray(Wo, dtype=np.float32)
    bo = np.asarray(bo, dtype=np.float32)

    cos_q, sin_q = _rope_tables(1024)
    cos_k, sin_k = _rope_tables(256)

    def blk(a, i):  # column block i (width 128) of a
        return a[:, i * 128:(i + 1) * 128]

    in_maps = []
    for c in range(8):
        b, j = c // 4, c % 4
        a0, a1 = 2 * j, 2 * j + 1
        g0 = 0 if j < 2 else 1

        xb = x[b]                                   # (S, D)
        xT3 = _chunk128(np.ascontiguousarray(xb.T))  # [128, 16, S]

        wq_sel = np.concatenate(
            [blk(Wq, a0), blk(Wq, a1), blk(Wq, a0 + 8), blk(Wq, a1 + 8)], axis=1)
        myblk = g0 + 2 * (j % 2)
        wk_sel = blk(Wk, myblk)
        wv_sel = blk(Wv, myblk)
        wo_sel = np.concatenate(
            [Wo[h * 128:(h + 1) * 128, :] for h in (a0, a1, a0 + 8, a1 + 8)],
            axis=0)

        cq_sel = _chunk128(np.ascontiguousarray(
            np.concatenate([blk(cos_q, a0), blk(cos_q, a1)], axis=1).T))
        sq_sel = _chunk128(np.ascontiguousarray(
            np.concatenate([blk(sin_q, a0), blk(sin_q, a1)], axis=1).T))
        ck_sel = np.ascontiguousarray(blk(cos_k, g0).T).astype(NPBF)
        sk_sel = np.ascontiguousarray(blk(sin_k, g0).T).astype(NPBF)

        in_maps.append({
            "xT": xT3,
            "wq": _chunk128(wq_sel),
            "wk": _chunk128(wk_sel),
            "wv": _chunk128(wv_sel),
            "wo": _chunk128(wo_sel),
            "cq": cq_sel, "sq": sq_sel, "ck": ck_sel, "sk": sk_sel,
        })

    global LAST_RESULT
    if _NC_CACHE is None:
        _NC_CACHE = _build_nc()
    res = run_bass_kernel_spmd(_NC_CACHE, in_maps, list(range(8)))
    LAST_RESULT = res

    partials = [
        res.results[c]["out"].astype(np.float32).transpose(1, 0, 2).reshape(S, D)
        for c in range(8)
    ]
    out = np.stack(
        [sum(partials[4 * b + j] for j in range(4)) for b in range(2)], axis=0
    )
    return (out + bo).astype(np.float32)


# revision 14
# speedup vs baseline: 1.2634x; 1.2634x over previous
"""GQA attention (b=2, s=2048, d=2048, H=16, Hkv=4, depth=128) on 8 trn2 cores.

Sharding: core c = 4*b + j (b in {0,1}, j in {0..3}) handles batch b and
q-heads {2j, 2j+1, 2j+8, 2j+9}.  This model's RoPE rotates the full projected
vector (pairing dim i with i + d/2), so roped q-head h mixes raw column
blocks {h mod 8, (h mod 8) + 8}; the head grouping above makes the Wq column
shard exactly 512 columns with no duplication.  Those q-heads attend kv-heads
{g0, g0+2} (g0 = 0 for j<2 else 1), which likewise pair up under RoPE.
Each core of a pair projects ONE raw k block and ONE v head; the pair swaps
them with a 2-way AllGather, halving the duplicated K/V projection work.
Wo is row-sharded over the 4 local head-dims; the 4 per-batch bf16 partials
are summed on the host (fp32) and bo added.

Device layout is fully transposed (feature dim on partitions): q_r^T, k_r^T
are [depth, s]; logits are computed as l^T = k_r^T.T @ q_r^T so the softmax
free axis is sq and the PV matmul needs no transposes (v is kept native
[s, dv] via on-chip DMA transposes).  All matmuls run in bf16 (fp32 PSUM).

Schedule (the PE is in-order, so overlap must be arranged at emission time):
attention runs sq-tile-major; each (st, head) does 8 iterations of
{2 QK matmuls -> one exp ACT over a [128,1024] 2-bank PSUM pair -> 2 PV
matmuls}.  The exp ACT (1114ns) outweighs the 4 matmuls (864ns), so the
output-projection matmuls of the PREVIOUS sq tile are interleaved between
iterations to keep the PE saturated (and its DVFS p-state at 2.4 GHz).
Softmax denominators are a bf16 running sum on DVE over the exp tiles, a
cross-partition ones-matmul reduce + broadcast on PE, and a DVE divide that
writes normalized o^T directly; the ACT engine does nothing but exp (one
activation table, loaded once).  PSUM drains run on ACT during the
projection phase and on DVE elsewhere (GPSIMD cannot touch PSUM).  Inputs stream per k-chunk so projection accumulation pipelines
with the DMAs; weights load as single DMAs to cut semaphore waits.
"""
import numpy as np
import ml_dtypes
from contextlib import ExitStack

import concourse.bass as bass
import concourse.mybir as mybir
import concourse.tile as tile
from concourse.bass import ts
from concourse.bass_utils import run_bass_kernel_spmd

BF = mybir.dt.bfloat16
F32 = mybir.dt.float32
NPBF = ml_dtypes.bfloat16

S = 2048          # sequence length
D = 2048          # d_model
DEPTH = 128       # head dim
NKC = 16          # contraction chunks of 128 over d_model
NST = 4           # 512-wide s tiles
INV_SQRT_D = 1.0 / float(np.sqrt(np.float32(DEPTH)))

_NC_CACHE = None
LAST_RESULT = None  # BassKernelResults of the most recent run (for profiling)


def _split_waits(nc, limit=1, dma_limit=1):
    """walrus rejects instructions carrying more than a couple of sem waits
    ('Too many sync wait commands'; DMA descriptors take only one).  Move
    excess waits onto dedicated NoOps on the same engine, placed immediately
    before the instruction."""
    idx = 0
    for f in nc.m.functions:
        for blk in f.blocks:
            insts = blk.instructions
            out = []
            for inst in insts:
                si = inst.sync_info
                lim = dma_limit if "DMA" in type(inst).__name__.upper() or \
                    "Dma" in type(inst).__name__ else limit
                if si is not None and len(si.on_wait) > lim:
                    waits = list(si.on_wait)
                    extra, keep = waits[:-lim], waits[-lim:]
                    for w in extra:
                        nop = mybir.InstNoOp(name=f"waitsplit_{idx}", ins=[], outs=[])
                        idx += 1
                        nop.engine = inst.engine
                        nop.bass_nofuse = True
                        nop.sync_info = mybir.SyncInfo(on_wait=[w], on_update=[])
                        out.append(nop)
                    inst.sync_info = mybir.SyncInfo(
                        on_wait=keep, on_update=list(si.on_update)
                    )
                out.append(inst)
            insts[:] = out


def _build_nc():
    nc = bass.Bass(num_devices=8)
    xT = nc.dram_tensor("xT", [128, NKC, S], BF, kind="ExternalInput")
    wq = nc.dram_tensor("wq", [128, NKC, 512], BF, kind="ExternalInput")
    wk = nc.dram_tensor("wk", [128, NKC, 128], BF, kind="ExternalInput")
    wv = nc.dram_tensor("wv", [128, NKC, 128], BF, kind="ExternalInput")
    wo = nc.dram_tensor("wo", [128, 4, D], BF, kind="ExternalInput")
    cq = nc.dram_tensor("cq", [128, 2, S], BF, kind="ExternalInput")
    sq = nc.dram_tensor("sq", [128, 2, S], BF, kind="ExternalInput")
    ck = nc.dram_tensor("ck", [128, S], BF, kind="ExternalInput")
    sk = nc.dram_tensor("sk", [128, S], BF, kind="ExternalInput")
    out = nc.dram_tensor("out", [128, 16, D], BF, kind="ExternalOutput")

    with tile.TileContext(nc) as tc, ExitStack() as top:
        pool_p = top.enter_context(tc.tile_pool(name="persist", bufs=1))
        # PSUM budget (8 banks): 4 banks for [128,1024] logit pairs /
        # projection pair-accs (tag psb, bufs=2), 1 bank for the PV
        # accumulator (tag pv, bufs=1), 2 banks for out-proj + KV proj accs
        # (tag ob, bufs=2), 1 bank for the denominator reduce + broadcast
        # (tag dn, sequential reuse).
        pp_big = top.enter_context(tc.tile_pool(name="psbig", bufs=2, space="PSUM"))
        pp_pv = top.enter_context(tc.tile_pool(name="pspv", bufs=1, space="PSUM"))
        pp_ob = top.enter_context(tc.tile_pool(name="psob", bufs=2, space="PSUM"))
        pp_dn = top.enter_context(tc.tile_pool(name="psdn", bufs=1, space="PSUM"))

        qr = pool_p.tile([128, 4, S], BF)        # roped qT, slots [a0,a1,a0+8,a1+8]
        kr = pool_p.tile([128, 2, S], BF)        # roped kT,  slots [g0, g0+2]
        vn = pool_p.tile([128, 2, NKC, DEPTH], BF)  # v native [p, g, skc, dv]
        wo_sb = pool_p.tile([128, 4, D], BF)
        ones_col_b = pool_p.tile([128, 1], BF)
        ones_row_b = pool_p.tile([1, 128], BF)
        nc.vector.memset(ones_col_b[:], 1.0)
        nc.vector.memset(ones_row_b[:], 1.0)

        # ---------------- phase 1: projections + rope -----------------
        with ExitStack() as p1:
            pool_x = p1.enter_context(tc.tile_pool(name="p1x", bufs=16))
            pool_w = p1.enter_context(tc.tile_pool(name="p1w", bufs=1))
            pool_tab = p1.enter_context(tc.tile_pool(name="p1t", bufs=1))
            pool_t = p1.enter_context(tc.tile_pool(name="p1tmp", bufs=3))
            pool_vt = p1.enter_context(tc.tile_pool(name="p1vt", bufs=1))
            pool_dram = p1.enter_context(tc.tile_pool(name="p1dram", bufs=1, space="DRAM"))

            # weights in single DMAs (fewer semaphores); x streams per chunk
            wk_sb = pool_w.tile([128, NKC, 128], BF)
            nc.sync.dma_start(wk_sb[:], wk[:])
            wv_sb = pool_w.tile([128, NKC, 128], BF)
            nc.sync.dma_start(wv_sb[:], wv[:])
            xTs = []
            for kc in range(NKC):
                xt_t = pool_x.tile([128, S], BF, tag="xt", name=f"xt_{kc}")
                nc.sync.dma_start(xt_t[:], xT[:, kc, :])
                xTs.append(xt_t)
            wq_sb = pool_w.tile([128, NKC, 512], BF)
            nc.sync.dma_start(wq_sb[:], wq[:])
            nc.sync.dma_start(wo_sb[:], wo[:])
            cq_sb = pool_tab.tile([128, 2, S], BF)
            sq_sb = pool_tab.tile([128, 2, S], BF)
            ck_sb = pool_tab.tile([128, S], BF)
            sk_sb = pool_tab.tile([128, S], BF)
            nc.sync.dma_start(ck_sb[:], ck[:])
            nc.sync.dma_start(sk_sb[:], sk[:])
            nc.sync.dma_start(cq_sb[:], cq[:])
            nc.sync.dma_start(sq_sb[:], sq[:])

            # K/V: each core of a pair projects ONE raw k block and ONE v
            # head; the pair exchanges them with an AllGather, then ropes /
            # transposes locally.  Halves the duplicated K/V projection work.
            kv_sb = pool_vt.tile([128, 2 * S], BF, tag="kvmine")
            for part, w_sel in ((0, wk_sb), (1, wv_sb)):
                for st in range(NST):
                    acc = pp_ob.tile([128, 512], F32, tag="ob")
                    for kc in range(NKC):
                        nc.tensor.matmul(
                            acc[:],
                            w_sel[:, kc, :],
                            xTs[kc][:, ts(st, 512)],
                            start=(kc == 0),
                            stop=(kc == NKC - 1),
                        )
                    nc.scalar.copy(kv_sb[:, ts(part * NST + st, 512)], acc[:])
            kv_in = pool_dram.tile([128, 2 * S], BF)
            kv_out = pool_dram.tile([2, 128, 2 * S], BF)
            nc.sync.dma_start(kv_in[:], kv_sb[:])
            nc.gpsimd.collective_compute(
                "AllGather",
                mybir.AluOpType.bypass,
                replica_groups=[[0, 1], [2, 3], [4, 5], [6, 7]],
                ins=[kv_in.opt()],
                outs=[kv_out.opt()],
            )
            kboth = pool_vt.tile([128, 2, S], BF, tag="kboth")
            vtboth = pool_vt.tile([128, 2, S], BF, tag="vtboth")
            for r in range(2):
                nc.sync.dma_start(kboth[:, r, :], kv_out[r, :, 0:S])
                nc.sync.dma_start(vtboth[:, r, :], kv_out[r, :, S:2 * S])

            # Q: raw blocks (i, 2+i) projected together into one 2-bank
            # PSUM pair, drained by a single Pool copy, roped on DVE.
            for i in range(2):
                for st in range(NST):
                    acc2 = pp_big.tile([128, 1024], F32, tag="psb")
                    for xb in range(2):
                        blk = i if xb == 0 else 2 + i
                        for kc in range(NKC):
                            nc.tensor.matmul(
                                acc2[:, ts(xb, 512)],
                                wq_sb[:, kc, ts(blk, 128)],
                                xTs[kc][:, ts(st, 512)],
                                start=(kc == 0),
                                stop=(kc == NKC - 1),
                            )
                    raw2 = pool_t.tile([128, 1024], BF, tag="raw")
                    nc.scalar.copy(raw2[:], acc2[:])
                    x1, x2 = raw2[:, 0:512], raw2[:, 512:1024]
                    c_ap = cq_sb[:, i, ts(st, 512)]
                    s_ap = sq_sb[:, i, ts(st, 512)]
                    t1 = pool_t.tile([128, 512], BF, tag="t1")
                    t2 = pool_t.tile([128, 512], BF, tag="t2")
                    nc.vector.tensor_mul(t1[:], x1, c_ap)
                    nc.vector.tensor_mul(t2[:], x2, s_ap)
                    nc.vector.tensor_sub(qr[:, i, ts(st, 512)], t1[:], t2[:])
                    t3 = pool_t.tile([128, 512], BF, tag="t1")
                    t4 = pool_t.tile([128, 512], BF, tag="t2")
                    nc.vector.tensor_mul(t3[:], x2, c_ap)
                    nc.vector.tensor_mul(t4[:], x1, s_ap)
                    nc.vector.tensor_add(qr[:, 2 + i, ts(st, 512)], t3[:], t4[:])

            # k rope from the gathered raw blocks (x1 = even core's block g0,
            # x2 = odd core's block g0+2)
            for st in range(NST):
                sl = ts(st, 512)
                x1, x2 = kboth[:, 0, sl], kboth[:, 1, sl]
                c_ap, s_ap = ck_sb[:, sl], sk_sb[:, sl]
                t1 = pool_t.tile([128, 512], BF, tag="t1")
                t2 = pool_t.tile([128, 512], BF, tag="t2")
                nc.vector.tensor_mul(t1[:], x1, c_ap)
                nc.vector.tensor_mul(t2[:], x2, s_ap)
                nc.vector.tensor_sub(kr[:, 0, sl], t1[:], t2[:])
                t3 = pool_t.tile([128, 512], BF, tag="t1")
                t4 = pool_t.tile([128, 512], BF, tag="t2")
                nc.vector.tensor_mul(t3[:], x2, c_ap)
                nc.vector.tensor_mul(t4[:], x1, s_ap)
                nc.vector.tensor_add(kr[:, 1, sl], t3[:], t4[:])

            # v native via DMA transpose
            for g in range(2):
                for skt in range(NKC):
                    nc.sync.dma_start_transpose(
                        vn[:, g, skt, :], vtboth[:, g, ts(skt, 128)]
                    )

        # ------------- phase 2: attention + output projection -------------
        with ExitStack() as p2:
            pool_exp = p2.enter_context(tc.tile_pool(name="exp", bufs=4))
            pool_sum = p2.enter_context(tc.tile_pool(name="sums", bufs=2))
            pool_pr = p2.enter_context(tc.tile_pool(name="pairs", bufs=3))
            pool_on = p2.enter_context(tc.tile_pool(name="onorm", bufs=2))
            pool_sm = p2.enter_context(tc.tile_pool(name="small", bufs=4))
            pool_osb = p2.enter_context(tc.tile_pool(name="osb", bufs=2))

            osb_state = {}

            def emit_outproj_group(onorm_prev, mprev, mi, ct):
                """4 accumulating matmuls of out[:, mprev, ct-block] over the
                4 local heads, drained to o_sb on DVE."""
                if ct == 0:
                    osb_state[mprev] = pool_osb.tile([128, D], BF, tag="osb",
                                                     name=f"osb_{mprev}")
                ob = pp_ob.tile([128, 512], F32, tag="ob", name=f"ob_{mprev}_{ct}")
                for hi2 in range(4):
                    nc.tensor.matmul(
                        ob[:],
                        onorm_prev[:, hi2, ts(mi, 128)],
                        wo_sb[:, hi2, ts(ct, 512)],
                        start=(hi2 == 0),
                        stop=(hi2 == 3),
                    )
                nc.vector.tensor_copy(osb_state[mprev][:, ts(ct, 512)], ob[:])
                if ct == 3:
                    nc.sync.dma_start(out[:, mprev, :], osb_state.pop(mprev)[:])

            onorm_prev = None
            for st in range(NST):
                onorm_st = pool_on.tile([128, 4, 512], BF, tag="on",
                                        name=f"on_{st}")
                for hi in range(4):
                    g = hi // 2
                    o_bank = pp_pv.tile([128, 512], F32, tag="pv",
                                        name=f"opv_{st}_{hi}")
                    fsum = pool_sum.tile([128, 1024], F32, tag="ss",
                                         name=f"fs_{st}_{hi}")
                    e_prev = None
                    pairs = []
                    for t in range(8):
                        lg = pp_big.tile([128, 1024], F32, tag="psb")
                        nc.tensor.matmul(
                            lg[:, 0:512],
                            kr[:, g, ts(2 * t, 128)],
                            qr[:, hi, ts(st, 512)],
                            start=True, stop=True,
                        )
                        nc.tensor.matmul(
                            lg[:, 512:1024],
                            kr[:, g, ts(2 * t + 1, 128)],
                            qr[:, hi, ts(st, 512)],
                            start=True, stop=True,
                        )
                        e = pool_exp.tile([128, 1024], BF, tag="exp",
                                          name=f"e_{st}_{hi}_{t}")
                        nc.scalar.activation(
                            e[:], lg[:],
                            mybir.ActivationFunctionType.Exp,
                            scale=INV_SQRT_D,
                        )
                        nc.tensor.matmul(
                            o_bank[:], vn[:, g, 2 * t, :], e[:, 0:512],
                            start=(t == 0), stop=False,
                        )
                        nc.tensor.matmul(
                            o_bank[:], vn[:, g, 2 * t + 1, :], e[:, 512:1024],
                            start=False, stop=(t == 7),
                        )
                        # denominator: bf16 pair-adds on DVE; the first two
                        # pair-pairs fold on the otherwise-idle Pool engine
                        # (SBUF-only ops), the last fold runs on DVE so den
                        # is ready quickly after the head ends
                        if t % 2 == 0:
                            e_prev = e
                        else:
                            p = pool_pr.tile([128, 1024], BF, tag="pr",
                                             name=f"p_{st}_{hi}_{t}")
                            nc.vector.tensor_add(p[:], e_prev[:], e[:])
                            pairs.append(p)
                            if t == 3:
                                nc.gpsimd.tensor_add(fsum[:], pairs[0][:],
                                                     pairs[1][:])
                            elif t == 5:
                                nc.gpsimd.tensor_add(fsum[:], fsum[:], p[:])
                        # keep the PE saturated while the ACT-bound attention
                        # iterations run: interleave the previous sq-tile's
                        # output projection between iterations
                        if onorm_prev is not None and t % 2 == 1:
                            emit_outproj_group(onorm_prev, 4 * (st - 1) + hi,
                                               hi, (t - 1) // 2)
                    fsum2 = pool_sum.tile([128, 1024], F32, tag="ss2",
                                          name=f"fs2_{st}_{hi}")
                    nc.vector.tensor_add(fsum2[:], fsum[:], pairs[3][:])
                    sden = pool_sm.tile([128, 512], BF, tag="sd")
                    nc.vector.tensor_add(sden[:], fsum2[:, 0:512],
                                         fsum2[:, 512:1024])
                    den = pp_dn.tile([1, 512], F32, tag="dn",
                                     name=f"dn_{st}_{hi}")
                    nc.tensor.matmul(den[:], ones_col_b[:], sden[:],
                                     start=True, stop=True)
                    lnd = pool_sm.tile([1, 512], F32, tag="db")
                    nc.scalar.activation(lnd[:], den[:],
                                         mybir.ActivationFunctionType.Ln)
                    rec = pool_sm.tile([1, 512], BF, tag="db")
                    nc.scalar.activation(rec[:], lnd[:],
                                         mybir.ActivationFunctionType.Exp,
                                         scale=-1.0)
                    bc = pp_dn.tile([128, 512], F32, tag="dn",
                                    name=f"bc_{st}_{hi}")
                    nc.tensor.matmul(bc[:], ones_row_b[:], rec[:],
                                     start=True, stop=True)
                    o_raw = pool_sm.tile([128, 512], BF, tag="or")
                    nc.vector.tensor_copy(o_raw[:], o_bank[:])
                    nc.vector.tensor_mul(onorm_st[:, hi, :], o_raw[:], bc[:])
                onorm_prev = onorm_st
            # last sq tile's output projection (nothing left to overlap)
            for mi in range(4):
                for ct in range(4):
                    emit_outproj_group(onorm_prev, 12 + mi, mi, ct)

    _split_waits(nc)
    return nc


def _chunk128(arr):
    """(K*128, N) f32 -> [128, K, N] bf16 with [p, k, n] = arr[k*128+p, n]."""
    k = arr.shape[0] // 128
    return np.ascontiguousarray(
        arr.reshape(k, 128, arr.shape[1]).transpose(1, 0, 2)
    ).astype(NPBF)


def _rope_tables(dim):
    pos = np.arange(S, dtype=np.float32)
    inv = (10000.0 ** (-(np.arange(dim, dtype=np.float32)) / np.float32(dim))
           ).astype(np.float32)
    freqs = pos[:, None] * inv[None, :]
    return np.cos(freqs).astype(np.float32), np.sin(freqs).astype(np.float32)


def kernel(x, mask, Wq, Wk, Wv, Wo, bo):
    global _NC_CACHE
    assert np.asarray(mask).all(), "kernel specialized for all-true mask"
    x = np.asarray(x, dtype=np.float32)
    Wq = np.asarray(Wq, dtype=np.float32)
    Wk = np.asarray(Wk, dtype=np.float32)
    Wv = np.asarray(Wv, dtype=np.float32)
    Wo = np.asarray(Wo, dtype=np.float32)
    bo = np.asarray(bo, dtype=np.float32)

    cos_q, sin_q = _rope_tables(1024)
    cos_k, sin_k = _rope_tables(256)

    def blk(a, i):  # column block i (width 128) of a
        return a[:, i * 128:(i + 1) * 128]

    in_maps = []
    for c in range(8):
        b, j = c // 4, c % 4
        a0, a1 = 2 * j, 2 * j + 1
        g0 = 0 if j < 2 else 1

        xb = x[b]                                   # (S, D)
        xT3 = _chunk128(np.ascontiguousarray(xb.T))  # [128, 16, S]

        wq_sel = np.concatenate(
            [blk(Wq, a0), blk(Wq, a1), blk(Wq, a0 + 8), blk(Wq, a1 + 8)], axis=1)
        myblk = g0 + 2 * (j % 2)
        wk_sel = blk(Wk, myblk)
        wv_sel = blk(Wv, myblk)
        wo_sel = np.concatenate(
            [Wo[h * 128:(h + 1) * 128, :] for h in (a0, a1, a0 + 8, a1 + 8)],
            axis=0)

        cq_sel = _chunk128(np.ascontiguousarray(
            np.concatenate([blk(cos_q, a0), blk(cos_q, a1)], axis=1).T))
        sq_sel = _chunk128(np.ascontiguousarray(
            np.concatenate([blk(sin_q, a0), blk(sin_q, a1)], axis=1).T))
        ck_sel = np.ascontiguousarray(blk(cos_k, g0).T).astype(NPBF)
        sk_sel = np.ascontiguousarray(blk(sin_k, g0).T).astype(NPBF)

        in_maps.append({
            "xT": xT3,
            "wq": _chunk128(wq_sel),
            "wk": _chunk128(wk_sel),
            "wv": _chunk128(wv_sel),
            "wo": _chunk128(wo_sel),
            "cq": cq_sel, "sq": sq_sel, "ck": ck_sel, "sk": sk_sel,
        })

    global LAST_RESULT
    if _NC_CACHE is None:
        _NC_CACHE = _build_nc()
    res = run_bass_kernel_spmd(_NC_CACHE, in_maps, list(range(8)))
    LAST_RESULT = res

    partials = [
        res.results[c]["out"].astype(np.float32).transpose(1, 0, 2).reshape(S, D)
        for c in range(8)
    ]
    out = np.stack(
        [sum(partials[4 * b + j] for j in range(4)) for b in range(2)], axis=0
    )
    return (out + bo).astype(np.float32)
